# revision 1
# baseline (speedup 1.0000x reference)
"""Multi-head attention (B=4, S=2048, D=1024, H=16, Dk=64) on 8 trn2 NeuronCores.

Sharding: core = (batch b, head-group g) with b in 0..3, g in 0..1.
Each core computes attention for its batch and its 8 heads, plus the partial
out-projection for its 512 columns of Wo.  Host sums the two partials per
batch and adds bo.

Per-core kernel (matmuls in float32r = TF32 fast mode, ~4e-4 rel err):
  phase A: PE-transpose q/k/v 128x128 tiles (f32r transpose mode); project to
           qhT/khT [c=512, s=2048] (c on partitions, pairs of heads per
           128-partition tile) and vh [s=2048, c] stored with a ones column
           per head ([128, 16, 8, 65] layout).  Biases are folded in as K=1
           outer-product matmuls opening each accumulation group.
  phase B (per 1024-wide query chunk, per head):
           scoresT[sk,sq] = khT_h^T @ qhT_h  (K=64 contraction)
           probsT = exp(scoresT/8 + maskbias[sk])   (mask folded into the
           ACT per-partition bias; masked keys underflow to exactly 0)
           attnT[c(+sums),sq] += vh_ext^T @ probsT  (ones column gives the
           softmax denominator in row 64 for free)
           normalize: approx-reciprocal of row 64, replicate across 64
           partitions with a K=1 outer-product matmul, multiply -> concatT
  phase C (interleaved per query chunk, after its 8 heads finish):
           out[sq,:] = concatT^T @ Wo  (accumulate 4 c-chunks in PSUM)
"""

import os
import sys

sys.path.insert(0, "/opt/trn_rl_repo")

import numpy as np

B, S, D, H, DK = 4, 2048, 1024, 16, 64
CPG = 512          # projection columns per core (8 heads x 64)
NCORES = 8

_cache = {}


def _build_nc():
    import concourse.bass as bass
    import concourse.tile as tile
    from concourse import bacc, mybir

    f32 = mybir.dt.float32
    R = mybir.dt.float32r
    Exp = mybir.ActivationFunctionType.Exp

    nc = bacc.Bacc("TRN2", target_bir_lowering=False, debug=False)

    q_d = nc.dram_tensor("q", [S, D], f32, kind="ExternalInput").ap()
    k_d = nc.dram_tensor("k", [S, D], f32, kind="ExternalInput").ap()
    v_d = nc.dram_tensor("v", [S, D], f32, kind="ExternalInput").ap()
    wq_d = nc.dram_tensor("wq", [D, CPG], R, kind="ExternalInput").ap()
    wk_d = nc.dram_tensor("wk", [D, CPG], R, kind="ExternalInput").ap()
    wv_d = nc.dram_tensor("wv", [D, CPG], R, kind="ExternalInput").ap()
    wo_d = nc.dram_tensor("wo", [CPG, D], R, kind="ExternalInput").ap()
    bq_d = nc.dram_tensor("bq", [CPG], R, kind="ExternalInput").ap()
    bk_d = nc.dram_tensor("bk", [CPG], R, kind="ExternalInput").ap()
    bv_d = nc.dram_tensor("bv", [CPG], R, kind="ExternalInput").ap()
    mb_d = nc.dram_tensor("maskbias", [128, 16], f32, kind="ExternalInput").ap()
    ones_d = nc.dram_tensor("ones", [128, 512], R, kind="ExternalInput").ap()
    ident_d = nc.dram_tensor("ident", [128, 128], f32, kind="ExternalInput").ap()
    out_d = nc.dram_tensor("out", [S, D], f32, kind="ExternalOutput").ap()

    NSQ = S // 512       # 4 query/key 512-blocks
    NDCH = D // 128      # 8 contraction chunks for projections
    NSK = S // 128       # 16 key chunks
    NPAIR = 4            # head pairs per core

    with tile.TileContext(nc) as tc:
        import contextlib

        with contextlib.ExitStack() as ctx:
            # ---------- persistent tensors + constants ----------
            persist = ctx.enter_context(tc.tile_pool(name="persist", bufs=1))
            consts = ctx.enter_context(tc.tile_pool(name="consts", bufs=1))

            qhT_sb = persist.tile([128, NPAIR, S], R)   # [c%128, pair, sq]
            khT_sb = persist.tile([128, NPAIR, S], R)
            vh_sb = persist.tile([128, NSK, 8, DK + 1], R)  # ones col at 64

            ones_sb = consts.tile([1, 512], R)
            nc.sync.dma_start(out=ones_sb, in_=ones_d[0:1, :])
            nc.sync.dma_start(
                out=vh_sb[:, :, :, DK],
                in_=ones_d[:, 0:128].rearrange("p (a b) -> p a b", a=16),
            )
            mb_sb = consts.tile([128, 16], f32)
            nc.sync.dma_start(out=mb_sb, in_=mb_d)
            wo_sb = consts.tile([128, NPAIR, D], R)
            for j in range(NPAIR):
                nc.sync.dma_start(
                    out=wo_sb[:, j, :], in_=wo_d[j * 128 : j * 128 + 128, :]
                )

            # ---------- phase A: transposes + projections ----------
            with contextlib.ExitStack() as actx:
                aconsts = actx.enter_context(tc.tile_pool(name="aconsts", bufs=1))
                wpool = actx.enter_context(tc.tile_pool(name="wpool", bufs=2))
                natpool = actx.enter_context(tc.tile_pool(name="natpool", bufs=6))
                xtpool = actx.enter_context(tc.tile_pool(name="xtpool", bufs=3))
                tppool = actx.enter_context(
                    tc.tile_pool(name="tppool", bufs=2, space="PSUM")
                )
                prpool = actx.enter_context(
                    tc.tile_pool(name="prpool", bufs=4, space="PSUM")
                )

                ident = aconsts.tile([128, 128], f32)
                nc.sync.dma_start(out=ident, in_=ident_d)
                bq_sb = aconsts.tile([1, CPG], R)
                nc.sync.dma_start(out=bq_sb, in_=bq_d[None, :])
                bk_sb = aconsts.tile([1, CPG], R)
                nc.sync.dma_start(out=bk_sb, in_=bk_d[None, :])
                bv_sb = aconsts.tile([1, CPG], R)
                nc.sync.dma_start(out=bv_sb, in_=bv_d[None, :])

                for tname, x_d, w_d, b_sb in (
                    ("q", q_d, wq_d, bq_sb),
                    ("k", k_d, wk_d, bk_sb),
                    ("v", v_d, wv_d, bv_sb),
                ):
                    w_sb = wpool.tile([128, NDCH, CPG], R, tag="w")
                    first_nats = []
                    if tname == "q":
                        for i in range(4):
                            x_nat = natpool.tile([128, D], f32, tag="nat")
                            nc.sync.dma_start(out=x_nat, in_=x_d[i * 128 : i * 128 + 128, :])
                            first_nats.append(x_nat)
                    for j in range(NDCH):
                        nc.sync.dma_start(
                            out=w_sb[:, j, :], in_=w_d[j * 128 : j * 128 + 128, :]
                        )
                    for sq in range(NSQ):
                        if sq == 0 and first_nats:
                            nats = first_nats
                        else:
                            nats = []
                            for i in range(4):
                                x_nat = natpool.tile([128, D], f32, tag="nat")
                                r0 = sq * 512 + i * 128
                                nc.sync.dma_start(out=x_nat, in_=x_d[r0 : r0 + 128, :])
                                nats.append(x_nat)

                        # open accumulation groups: bias outer-product first
                        prs = []
                        for cch in range(4):
                            pr = prpool.tile([128, 512], f32, tag="pr")
                            prs.append(pr)
                            if tname == "v":
                                nc.tensor.matmul(
                                    pr,
                                    lhsT=ones_sb[0:1, 0:128],
                                    rhs=b_sb[0:1, :],
                                    start=True,
                                    stop=False,
                                )
                            else:
                                nc.tensor.matmul(
                                    pr,
                                    lhsT=b_sb[0:1, cch * 128 : cch * 128 + 128],
                                    rhs=ones_sb[0:1, 0:512],
                                    start=True,
                                    stop=False,
                                )

                        for j in range(NDCH):
                            tp = tppool.tile([128, 512], f32, tag="tp")
                            for i in range(4):
                                nc.tensor.transpose(
                                    out=tp[:, i * 128 : i * 128 + 128],
                                    in_=nats[i][:, j * 128 : j * 128 + 128],
                                    identity=ident,
                                )
                            xt = xtpool.tile([128, 512], R, tag="xt")
                            nc.scalar.copy(out=xt, in_=tp)
                            for cch in range(4):
                                if tname == "v":
                                    # vh[sk,c]: lhsT = xT chunk, rhs = W chunk
                                    nc.tensor.matmul(
                                        prs[cch],
                                        lhsT=xt[:, cch * 128 : cch * 128 + 128],
                                        rhs=w_sb[:, j, :],
                                        start=False,
                                        stop=(j == NDCH - 1),
                                    )
                                else:
                                    # qhT[c,sq]: lhsT = W chunk, rhs = xT
                                    nc.tensor.matmul(
                                        prs[cch],
                                        lhsT=w_sb[:, j, cch * 128 : cch * 128 + 128],
                                        rhs=xt,
                                        start=False,
                                        stop=(j == NDCH - 1),
                                    )

                        for cch in range(4):
                            if tname == "v":
                                skc = sq * 4 + cch
                                nc.vector.tensor_copy(
                                    out=vh_sb[:, skc, :, 0:DK],
                                    in_=prs[cch].rearrange("p (h d) -> p h d", h=8),
                                )
                            else:
                                dst = qhT_sb if tname == "q" else khT_sb
                                nc.vector.tensor_copy(
                                    out=dst[:, cch, sq * 512 : sq * 512 + 512],
                                    in_=prs[cch],
                                )

            # ---------- phase B: attention ----------
            concpool = ctx.enter_context(tc.tile_pool(name="concpool", bufs=1))
            concatT_sb = concpool.tile([128, NPAIR, S], R)
            with contextlib.ExitStack() as bctx:
                probpool = bctx.enter_context(tc.tile_pool(name="probpool", bufs=3))
                npool = bctx.enter_context(tc.tile_pool(name="npool", bufs=2))
                rppool = bctx.enter_context(tc.tile_pool(name="rppool", bufs=1))
                scpool = bctx.enter_context(
                    tc.tile_pool(name="scpool", bufs=2, space="PSUM")
                )
                atpool = bctx.enter_context(
                    tc.tile_pool(name="atpool", bufs=3, space="PSUM")
                )
                reppool = bctx.enter_context(
                    tc.tile_pool(name="reppool", bufs=1, space="PSUM")
                )

                for sq2 in range(S // 1024):
                    for pair in range(NPAIR):
                        for hh in range(2):
                            h = pair * 2 + hh
                            base = hh * 64
                            at_ps0 = atpool.tile([128, 512], f32, tag="at")
                            at_ps1 = atpool.tile([128, 512], f32, tag="at")
                            at_halves = (at_ps0, at_ps1)
                            for sk in range(NSK):
                                sc_ps = scpool.tile([128, 1024], f32, tag="sc")
                                for half in range(2):
                                    qoff = sq2 * 1024 + half * 512
                                    nc.tensor.matmul(
                                        sc_ps[:, half * 512 : half * 512 + 512],
                                        lhsT=khT_sb[
                                            base : base + 64,
                                            pair,
                                            sk * 128 : sk * 128 + 128,
                                        ],
                                        rhs=qhT_sb[
                                            base : base + 64, pair, qoff : qoff + 512
                                        ],
                                        start=True,
                                        stop=True,
                                    )
                                probs = probpool.tile([128, 1024], R, tag="probs")
                                nc.scalar.activation(
                                    out=probs,
                                    in_=sc_ps,
                                    func=Exp,
                                    bias=mb_sb[:, sk : sk + 1],
                                    scale=0.125,
                                )
                                for half in range(2):
                                    nc.tensor.matmul(
                                        at_halves[half][0:65, :],
                                        lhsT=vh_sb[:, sk, h, :],
                                        rhs=probs[:, half * 512 : half * 512 + 512],
                                        start=(sk == 0),
                                        stop=(sk == NSK - 1),
                                    )
                            attn_sb = npool.tile([128, 1024], f32, tag="attn")
                            for half in range(2):
                                nc.vector.tensor_copy(
                                    out=attn_sb[
                                        0:65, half * 512 : half * 512 + 512
                                    ],
                                    in_=at_halves[half][0:65, :],
                                )
                            recip32 = rppool.tile([1, 1024], f32, tag="recip32")
                            nc.vector.reciprocal(recip32, attn_sb[64:65, :])
                            recip = rppool.tile([1, 1024], R, tag="recip")
                            nc.vector.tensor_copy(out=recip, in_=recip32)
                            for half in range(2):
                                rep_ps = reppool.tile([64, 512], f32, tag="rep")
                                nc.tensor.matmul(
                                    rep_ps,
                                    lhsT=ones_sb[0:1, 0:64],
                                    rhs=recip[0:1, half * 512 : half * 512 + 512],
                                    start=True,
                                    stop=True,
                                )
                                nc.vector.tensor_mul(
                                    concatT_sb[
                                        base : base + 64,
                                        pair,
                                        sq2 * 1024 + half * 512 : sq2 * 1024
                                        + half * 512
                                        + 512,
                                    ],
                                    attn_sb[0:64, half * 512 : half * 512 + 512],
                                    rep_ps,
                                )

            # ---------- phase C: out projection ----------
            with contextlib.ExitStack() as cctx:
                outpool = cctx.enter_context(
                    tc.tile_pool(name="outpool", bufs=3)
                )
                opspool = cctx.enter_context(
                    tc.tile_pool(name="opspool", bufs=4, space="PSUM")
                )
                for sqc in range(S // 128):
                    for do in range(2):
                        o_ps = opspool.tile([128, 512], f32, tag="ops")
                        for j in range(NPAIR):
                            nc.tensor.matmul(
                                o_ps,
                                lhsT=concatT_sb[
                                    :, j, sqc * 128 : sqc * 128 + 128
                                ],
                                rhs=wo_sb[:, j, do * 512 : do * 512 + 512],
                                start=(j == 0),
                                stop=(j == NPAIR - 1),
                            )
                        o_sb = outpool.tile([128, 512], f32, tag="osb")
                        nc.vector.tensor_copy(out=o_sb, in_=o_ps)
                        nc.sync.dma_start(
                            out=out_d[
                                sqc * 128 : sqc * 128 + 128,
                                do * 512 : do * 512 + 512,
                            ],
                            in_=o_sb,
                        )

    nc.compile()
    return nc


def get_nc():
    if "nc" not in _cache:
        _cache["nc"] = _build_nc()
    return _cache["nc"]


def make_in_maps(q, k, v, mask, Wq, bq, Wk, bk, Wv, bv, Wo, bo):
    f32 = np.float32
    c = np.ascontiguousarray
    in_maps = []
    for core in range(NCORES):
        b, g = core // 2, core % 2
        cols = slice(g * CPG, (g + 1) * CPG)
        mb = (-1e9 * (1.0 - np.asarray(mask[b, 0], f32))).reshape(16, 128).T
        in_maps.append(
            {
                "q": c(np.asarray(q[b], f32)),
                "k": c(np.asarray(k[b], f32)),
                "v": c(np.asarray(v[b], f32)),
                "wq": c(np.asarray(Wq[:, cols], f32)),
                "wk": c(np.asarray(Wk[:, cols], f32)),
                "wv": c(np.asarray(Wv[:, cols], f32)),
                "wo": c(np.asarray(Wo[cols, :], f32)),
                "bq": c(np.asarray(bq[cols], f32)),
                "bk": c(np.asarray(bk[cols], f32)),
                "bv": c(np.asarray(bv[cols], f32)),
                "maskbias": c(mb),
                "ones": np.ones((128, 512), f32),
                "ident": np.eye(128, dtype=f32),
            }
        )
    return in_maps


def gather(results, bo):
    out = np.zeros((B, S, D), np.float32)
    for core in range(NCORES):
        b = core // 2
        out[b] += results[core]["out"]
    out += np.asarray(bo, np.float32)[None, None, :]
    return out


def run_on_hw(in_maps, trace=False, trace_cores=None):
    from concourse.bass_utils import run_bass_kernel_spmd

    nc = get_nc()
    return run_bass_kernel_spmd(
        nc,
        in_maps,
        list(range(NCORES)),
        trace=trace,
        trace_cores=trace_cores,
    )


def kernel(q, k, v, mask, Wq, bq, Wk, bk, Wv, bv, Wo, bo):
    in_maps = make_in_maps(q, k, v, mask, Wq, bq, Wk, bk, Wv, bv, Wo, bo)
    res = run_on_hw(in_maps)
    return gather(res.results, bo)



# revision 8
# speedup vs baseline: 1.1569x; 1.1569x over previous
"""Multi-head attention (B=4, S=2048, D=1024, H=16, Dk=64) on 8 trn2 NeuronCores.

Sharding: core = (batch b, head-group g) with b in 0..3, g in 0..1.
Each core computes attention for its batch and its 8 heads, plus the partial
out-projection for its 512 columns of Wo.  Host sums the two partials per
batch and adds bo.

Per-core kernel (matmuls in bf16, f32 PSUM accumulation):
  phase A: PE-transpose q/k/v 128x128 tiles (bf16 transpose mode); project to
           qhT/khT [c=512, s=2048] (c on partitions, pairs of heads per
           128-partition tile) and vh [s=2048, c] stored with a ones column
           per head ([128, 16, 8, 65] layout).  Biases are folded in as K=1
           outer-product matmuls opening each accumulation group.
  phase B (per 1024-wide query chunk, per head):
           scoresT[sk,sq] = khT_h^T @ qhT_h  (K=64 contraction)
           probsT = exp(scoresT/8 + maskbias[sk])   (mask folded into the
           ACT per-partition bias; masked keys underflow to exactly 0)
           attnT[c(+sums),sq] += vh_ext^T @ probsT  (ones column gives the
           softmax denominator in row 64 for free)
           normalize: approx-reciprocal of row 64, replicate across 64
           partitions with a K=1 outer-product matmul, multiply -> concatT
  phase C (interleaved per query chunk, after its 8 heads finish):
           out[sq,:] = concatT^T @ Wo  (accumulate 4 c-chunks in PSUM)
"""

import os
import sys

sys.path.insert(0, "/opt/trn_rl_repo")

import numpy as np

B, S, D, H, DK = 4, 2048, 1024, 16, 64
CPG = 512          # projection columns per core (8 heads x 64)
NCORES = 8

_cache = {}


def _build_nc():
    import concourse.bass as bass
    import concourse.tile as tile
    from concourse import bacc, mybir

    f32 = mybir.dt.float32
    bf16 = mybir.dt.bfloat16
    Exp = mybir.ActivationFunctionType.Exp

    nc = bacc.Bacc("TRN2", target_bir_lowering=False, debug=False)

    q_d = nc.dram_tensor("q", [S, D], bf16, kind="ExternalInput").ap()
    k_d = nc.dram_tensor("k", [S, D], bf16, kind="ExternalInput").ap()
    v_d = nc.dram_tensor("v", [S, D], bf16, kind="ExternalInput").ap()
    wq_d = nc.dram_tensor("wq", [D, CPG], bf16, kind="ExternalInput").ap()
    wk_d = nc.dram_tensor("wk", [D, CPG], bf16, kind="ExternalInput").ap()
    wv_d = nc.dram_tensor("wv", [D, CPG], bf16, kind="ExternalInput").ap()
    wo_d = nc.dram_tensor("wo", [CPG, D], bf16, kind="ExternalInput").ap()
    bq_d = nc.dram_tensor("bq", [CPG], bf16, kind="ExternalInput").ap()
    bk_d = nc.dram_tensor("bk", [CPG], bf16, kind="ExternalInput").ap()
    bv_d = nc.dram_tensor("bv", [CPG], bf16, kind="ExternalInput").ap()
    mb_d = nc.dram_tensor("maskbias", [128, 16], f32, kind="ExternalInput").ap()
    ones_d = nc.dram_tensor("ones", [128, 512], bf16, kind="ExternalInput").ap()
    ident_d = nc.dram_tensor("ident", [128, 128], bf16, kind="ExternalInput").ap()
    out_d = nc.dram_tensor("out", [S, D], f32, kind="ExternalOutput").ap()

    NSQ = S // 512       # 4 query/key 512-blocks
    NDCH = D // 128      # 8 contraction chunks for projections
    NSK = S // 128       # 16 key chunks
    NPAIR = 4            # head pairs per core

    with tile.TileContext(nc) as tc:
        import contextlib

        with contextlib.ExitStack() as ctx:
            # ---------- persistent tensors + constants ----------
            persist = ctx.enter_context(tc.tile_pool(name="persist", bufs=1))
            consts = ctx.enter_context(tc.tile_pool(name="consts", bufs=1))

            qhT_sb = persist.tile([128, NPAIR, S], bf16)   # [c%128, pair, sq]
            khT_sb = persist.tile([128, NPAIR, S], bf16)
            vh_sb = persist.tile([128, NSK, 8, DK + 1], bf16)  # ones col at 64

            ones_sb = consts.tile([1, 512], bf16)
            nc.sync.dma_start(out=ones_sb, in_=ones_d[0:1, :])
            nc.sync.dma_start(
                out=vh_sb[:, :, :, DK],
                in_=ones_d[:, 0:128].rearrange("p (a b) -> p a b", a=16),
            )
            mb_sb = consts.tile([128, 16], f32)
            nc.sync.dma_start(out=mb_sb, in_=mb_d)
            wo_sb = consts.tile([128, NPAIR, D], bf16)
            for j in range(NPAIR):
                nc.sync.dma_start(
                    out=wo_sb[:, j, :], in_=wo_d[j * 128 : j * 128 + 128, :]
                )

            # ---------- phase A: transposes + projections ----------
            with contextlib.ExitStack() as actx:
                aconsts = actx.enter_context(tc.tile_pool(name="aconsts", bufs=1))
                wpool = actx.enter_context(tc.tile_pool(name="wpool", bufs=2))
                natpool = actx.enter_context(tc.tile_pool(name="natpool", bufs=6))
                xtpool = actx.enter_context(tc.tile_pool(name="xtpool", bufs=3))
                tppool = actx.enter_context(
                    tc.tile_pool(name="tppool", bufs=2, space="PSUM")
                )
                prpool = actx.enter_context(
                    tc.tile_pool(name="prpool", bufs=4, space="PSUM")
                )

                ident = aconsts.tile([128, 128], bf16)
                nc.sync.dma_start(out=ident, in_=ident_d)
                bq_sb = aconsts.tile([1, CPG], bf16)
                nc.sync.dma_start(out=bq_sb, in_=bq_d[None, :])
                bk_sb = aconsts.tile([1, CPG], bf16)
                nc.sync.dma_start(out=bk_sb, in_=bk_d[None, :])
                bv_sb = aconsts.tile([1, CPG], bf16)
                nc.sync.dma_start(out=bv_sb, in_=bv_d[None, :])

                for tname, x_d, w_d, b_sb in (
                    ("q", q_d, wq_d, bq_sb),
                    ("k", k_d, wk_d, bk_sb),
                    ("v", v_d, wv_d, bv_sb),
                ):
                    w_sb = wpool.tile([128, NDCH, CPG], bf16, tag="w")
                    first_nats = []
                    if tname == "q":
                        for i in range(4):
                            x_nat = natpool.tile([128, D], bf16, tag="nat")
                            nc.sync.dma_start(out=x_nat, in_=x_d[i * 128 : i * 128 + 128, :])
                            first_nats.append(x_nat)
                    for j in range(NDCH):
                        nc.sync.dma_start(
                            out=w_sb[:, j, :], in_=w_d[j * 128 : j * 128 + 128, :]
                        )
                    for sq in range(NSQ):
                        if sq == 0 and first_nats:
                            nats = first_nats
                        else:
                            nats = []
                            for i in range(4):
                                x_nat = natpool.tile([128, D], bf16, tag="nat")
                                r0 = sq * 512 + i * 128
                                nc.sync.dma_start(out=x_nat, in_=x_d[r0 : r0 + 128, :])
                                nats.append(x_nat)

                        # open accumulation groups: bias outer-product first
                        prs = []
                        for cch in range(4):
                            pr = prpool.tile([128, 512], f32, tag="pr")
                            prs.append(pr)
                            if tname == "v":
                                nc.tensor.matmul(
                                    pr,
                                    lhsT=ones_sb[0:1, 0:128],
                                    rhs=b_sb[0:1, :],
                                    start=True,
                                    stop=False,
                                )
                            else:
                                nc.tensor.matmul(
                                    pr,
                                    lhsT=b_sb[0:1, cch * 128 : cch * 128 + 128],
                                    rhs=ones_sb[0:1, 0:512],
                                    start=True,
                                    stop=False,
                                )

                        for j in range(NDCH):
                            # full PSUM bank (1024 bf16): matmul PSUM writes are
                            # bank-granular on HW; a half-bank tile gets clobbered
                            tp_bank = tppool.tile([128, 1024], bf16, tag="tp")
                            tp = tp_bank[:, 0:512]
                            for i in range(4):
                                nc.tensor.transpose(
                                    out=tp[:, i * 128 : i * 128 + 128],
                                    in_=nats[i][:, j * 128 : j * 128 + 128],
                                    identity=ident,
                                )
                            xt = xtpool.tile([128, 512], bf16, tag="xt")
                            nc.vector.tensor_copy(out=xt, in_=tp)
                            for cch in range(4):
                                if tname == "v":
                                    # vh[sk,c]: lhsT = xT chunk, rhs = W chunk
                                    nc.tensor.matmul(
                                        prs[cch],
                                        lhsT=xt[:, cch * 128 : cch * 128 + 128],
                                        rhs=w_sb[:, j, :],
                                        start=False,
                                        stop=(j == NDCH - 1),
                                    )
                                else:
                                    # qhT[c,sq]: lhsT = W chunk, rhs = xT
                                    nc.tensor.matmul(
                                        prs[cch],
                                        lhsT=w_sb[:, j, cch * 128 : cch * 128 + 128],
                                        rhs=xt,
                                        start=False,
                                        stop=(j == NDCH - 1),
                                    )

                        for cch in range(4):
                            if tname == "v":
                                skc = sq * 4 + cch
                                nc.vector.tensor_copy(
                                    out=vh_sb[:, skc, :, 0:DK],
                                    in_=prs[cch].rearrange("p (h d) -> p h d", h=8),
                                )
                            else:
                                dst = qhT_sb if tname == "q" else khT_sb
                                nc.vector.tensor_copy(
                                    out=dst[:, cch, sq * 512 : sq * 512 + 512],
                                    in_=prs[cch],
                                )

            # ---------- phase B: attention ----------
            # Software-pipelined: per sk, emit score(sk) then attn(sk-1) so
            # the PE queue always has an independent score matmul to run
            # while the ACT engine computes exp(sk).  The normalization tail
            # of head-chunk i-1 (reciprocal chain + rep/mul) is hoisted into
            # chunk i's sk loop to hide its latency.
            concpool = ctx.enter_context(tc.tile_pool(name="concpool", bufs=1))
            concatT_sb = concpool.tile([128, NPAIR, S], bf16)
            with contextlib.ExitStack() as bctx:
                probpool = bctx.enter_context(tc.tile_pool(name="probpool", bufs=3))
                rppool = bctx.enter_context(tc.tile_pool(name="rppool", bufs=2))
                scpool = bctx.enter_context(
                    tc.tile_pool(name="scpool", bufs=2, space="PSUM")
                )
                atpool = bctx.enter_context(
                    tc.tile_pool(name="atpool", bufs=2, space="PSUM")
                )

                chunks = [
                    (sq2, pair, hh)
                    for sq2 in range(S // 1024)
                    for pair in range(NPAIR)
                    for hh in range(2)
                ]

                def emit_norm(state):
                    """Normalization tail for a finished head-chunk: denom ->
                    1/denom (p0), broadcast via K=1 matmul into a borrowed sc
                    bank, multiply PSUM attn rows into concatT."""
                    (sq2, pair, hh), at_ps = state
                    base = hh * 64
                    # custom-DVE recip needs partition-0 input: native copy
                    # shifts the denominator row 64 -> 0
                    den0 = rppool.tile([1, 1024], f32, tag="den0")
                    nc.vector.tensor_copy(out=den0, in_=at_ps[64:65, :])
                    recip32 = rppool.tile([1, 1024], f32, tag="recip32")
                    nc.vector.reciprocal_approx_fast(out=recip32, in_=den0)
                    recip = rppool.tile([1, 1024], bf16, tag="recip")
                    nc.vector.tensor_copy(out=recip, in_=recip32)
                    # DVE can read only one PSUM operand per op: stage attn
                    attn_sb = rppool.tile([64, 1024], f32, tag="attn")
                    nc.vector.tensor_copy(out=attn_sb, in_=at_ps[0:64, :])
                    for half in range(2):
                        rep_bank = scpool.tile([128, 1024], f32, tag="sc")
                        rep_ps = rep_bank[0:64, 0:512]
                        nc.tensor.matmul(
                            rep_ps,
                            lhsT=ones_sb[0:1, 0:64],
                            rhs=recip[0:1, half * 512 : half * 512 + 512],
                            start=True,
                            stop=True,
                        )
                        nc.vector.tensor_mul(
                            concatT_sb[
                                base : base + 64,
                                pair,
                                sq2 * 1024 + half * 512 : sq2 * 1024
                                + half * 512
                                + 512,
                            ],
                            attn_sb[:, half * 512 : half * 512 + 512],
                            rep_ps,
                        )

                pending = None
                for ci, (sq2, pair, hh) in enumerate(chunks):
                    h = pair * 2 + hh
                    base = hh * 64
                    at_ps = atpool.tile([128, 1024], f32, tag="at")
                    prev_probs = None
                    for sk in range(NSK):
                        sc_ps = scpool.tile([128, 1024], f32, tag="sc")
                        for half in range(2):
                            qoff = sq2 * 1024 + half * 512
                            nc.tensor.matmul(
                                sc_ps[:, half * 512 : half * 512 + 512],
                                lhsT=khT_sb[
                                    base : base + 64,
                                    pair,
                                    sk * 128 : sk * 128 + 128,
                                ],
                                rhs=qhT_sb[
                                    base : base + 64, pair, qoff : qoff + 512
                                ],
                                start=True,
                                stop=True,
                            )
                        probs = probpool.tile([128, 1024], bf16, tag="probs")
                        nc.scalar.activation(
                            out=probs,
                            in_=sc_ps,
                            func=Exp,
                            bias=mb_sb[:, sk : sk + 1],
                            scale=0.125,
                        )
                        if prev_probs is not None:
                            psk, pp = prev_probs
                            for half in range(2):
                                nc.tensor.matmul(
                                    at_ps[0:65, half * 512 : half * 512 + 512],
                                    lhsT=vh_sb[:, psk, h, :],
                                    rhs=pp[:, half * 512 : half * 512 + 512],
                                    start=(psk == 0),
                                    stop=False,
                                )
                        prev_probs = (sk, probs)
                        if sk == 2 and pending is not None:
                            emit_norm(pending)
                            pending = None
                    psk, pp = prev_probs
                    for half in range(2):
                        nc.tensor.matmul(
                            at_ps[0:65, half * 512 : half * 512 + 512],
                            lhsT=vh_sb[:, psk, h, :],
                            rhs=pp[:, half * 512 : half * 512 + 512],
                            start=False,
                            stop=True,
                        )
                    pending = ((sq2, pair, hh), at_ps)
                emit_norm(pending)
            # ---------- phase C: out projection ----------
            with contextlib.ExitStack() as cctx:
                outpool = cctx.enter_context(
                    tc.tile_pool(name="outpool", bufs=3)
                )
                opspool = cctx.enter_context(
                    tc.tile_pool(name="opspool", bufs=4, space="PSUM")
                )
                for sqc in range(S // 128):
                    for do in range(2):
                        o_ps = opspool.tile([128, 512], f32, tag="ops")
                        for j in range(NPAIR):
                            nc.tensor.matmul(
                                o_ps,
                                lhsT=concatT_sb[
                                    :, j, sqc * 128 : sqc * 128 + 128
                                ],
                                rhs=wo_sb[:, j, do * 512 : do * 512 + 512],
                                start=(j == 0),
                                stop=(j == NPAIR - 1),
                            )
                        o_sb = outpool.tile([128, 512], f32, tag="osb")
                        nc.vector.tensor_copy(out=o_sb, in_=o_ps)
                        nc.sync.dma_start(
                            out=out_d[
                                sqc * 128 : sqc * 128 + 128,
                                do * 512 : do * 512 + 512,
                            ],
                            in_=o_sb,
                        )

    nc.compile()
    return nc


def get_nc():
    if "nc" not in _cache:
        _cache["nc"] = _build_nc()
    return _cache["nc"]


def make_in_maps(q, k, v, mask, Wq, bq, Wk, bk, Wv, bv, Wo, bo):
    import ml_dtypes

    f32 = np.float32
    bf16 = ml_dtypes.bfloat16
    c = np.ascontiguousarray
    in_maps = []
    for core in range(NCORES):
        b, g = core // 2, core % 2
        cols = slice(g * CPG, (g + 1) * CPG)
        mb = (-1e9 * (1.0 - np.asarray(mask[b, 0], f32))).reshape(16, 128).T
        in_maps.append(
            {
                "q": c(np.asarray(q[b], f32).astype(bf16)),
                "k": c(np.asarray(k[b], f32).astype(bf16)),
                "v": c(np.asarray(v[b], f32).astype(bf16)),
                "wq": c(np.asarray(Wq[:, cols], f32).astype(bf16)),
                "wk": c(np.asarray(Wk[:, cols], f32).astype(bf16)),
                "wv": c(np.asarray(Wv[:, cols], f32).astype(bf16)),
                "wo": c(np.asarray(Wo[cols, :], f32).astype(bf16)),
                "bq": c(np.asarray(bq[cols], f32).astype(bf16)),
                "bk": c(np.asarray(bk[cols], f32).astype(bf16)),
                "bv": c(np.asarray(bv[cols], f32).astype(bf16)),
                "maskbias": c(mb),
                "ones": np.ones((128, 512), bf16),
                "ident": np.eye(128, dtype=bf16),
            }
        )
    return in_maps


def gather(results, bo):
    out = np.zeros((B, S, D), np.float32)
    for core in range(NCORES):
        b = core // 2
        out[b] += results[core]["out"]
    out += np.asarray(bo, np.float32)[None, None, :]
    return out


def run_on_hw(in_maps, trace=False, trace_cores=None):
    from concourse.bass_utils import run_bass_kernel_spmd

    nc = get_nc()
    return run_bass_kernel_spmd(
        nc,
        in_maps,
        list(range(NCORES)),
        trace=trace,
        trace_cores=trace_cores,
    )


def kernel(q, k, v, mask, Wq, bq, Wk, bk, Wv, bv, Wo, bo):
    in_maps = make_in_maps(q, k, v, mask, Wq, bq, Wk, bk, Wv, bv, Wo, bo)
    res = run_on_hw(in_maps)
    return gather(res.results, bo)


# revision 17
# speedup vs baseline: 1.3367x; 1.1554x over previous
"""Multi-head attention (B=4, S=2048, D=1024, H=16, Dk=64) on 8 trn2 NeuronCores.

Sharding: core = (batch b, head-group g) with b in 0..3, g in 0..1.
Each core computes attention for its batch and its 8 heads, plus the partial
out-projection for its 512 columns of Wo.  Host sums the two partials per
batch and adds bo.

Per-core kernel (matmuls bf16, f32 PSUM accumulation):
  phase A: PE-transpose q/k/v 128x128 tiles (bf16, 2 d-chunks per PSUM bank)
           and project into f32 PSUM; qhT/khT stored bf16 [c=512, s=2048]
           (c on partitions, head pairs per 128-partition tile), vh stored
           bf16 [s, c] with a ones column per head ([128, 16, 8, 65]).
           Biases fold in as K=1 outer-product matmuls opening each group.
  phase B (per 1024-wide query chunk, per head), software-pipelined so the
           PE always has a score matmul to run while ACT computes exp:
           scoresT[sk,sq] = khT_h^T @ qhT_h  (K=64)
           probsT = exp(scoresT/8 + maskbias[sk]) -> bf16 (mask folds into
           the ACT bias; masked keys underflow to exactly 0)
           attnT[c(+sum),sq] += vh^T @ probsT (ones column accumulates the
           softmax denominator in row 64)
           normalize: denom row -> partition 0, fast approx reciprocal,
           broadcast with a K=1 matmul into a borrowed sc bank, multiply.
  phase C: out[sq,:] = concatT^T @ Wo  (accumulate 4 c-chunks in PSUM)
"""

import os
import sys

sys.path.insert(0, "/opt/trn_rl_repo")

import numpy as np

B, S, D, H, DK = 4, 2048, 1024, 16, 64
CPG = 512          # projection columns per core (8 heads x 64)
NCORES = 8

_cache = {}


def _build_nc():
    import concourse.bass as bass
    import concourse.tile as tile
    from concourse import bacc, mybir

    f32 = mybir.dt.float32
    bf16 = mybir.dt.bfloat16
    Exp = mybir.ActivationFunctionType.Exp

    nc = bacc.Bacc("TRN2", target_bir_lowering=False, debug=False)

    q_d = nc.dram_tensor("q", [S, D], bf16, kind="ExternalInput").ap()
    k_d = nc.dram_tensor("k", [S, D], bf16, kind="ExternalInput").ap()
    v_d = nc.dram_tensor("v", [S, D], bf16, kind="ExternalInput").ap()
    wq_d = nc.dram_tensor("wq", [D, CPG], bf16, kind="ExternalInput").ap()
    wk_d = nc.dram_tensor("wk", [D, CPG], bf16, kind="ExternalInput").ap()
    wv_d = nc.dram_tensor("wv", [D, CPG], bf16, kind="ExternalInput").ap()
    wo_d = nc.dram_tensor("wo", [CPG, D], bf16, kind="ExternalInput").ap()
    bq_d = nc.dram_tensor("bq", [CPG], bf16, kind="ExternalInput").ap()
    bk_d = nc.dram_tensor("bk", [CPG], bf16, kind="ExternalInput").ap()
    bv_d = nc.dram_tensor("bv", [CPG], bf16, kind="ExternalInput").ap()
    mb_d = nc.dram_tensor("maskbias", [128, 16], f32, kind="ExternalInput").ap()
    ones_d = nc.dram_tensor("ones", [128, 512], bf16, kind="ExternalInput").ap()
    ident_d = nc.dram_tensor("ident", [128, 128], bf16, kind="ExternalInput").ap()
    out_d = nc.dram_tensor("out", [S, D], f32, kind="ExternalOutput").ap()

    NSQ = S // 512       # 4 query/key 512-blocks
    NDCH = D // 128      # 8 contraction chunks for projections
    NJP = NDCH // 2      # transpose bank pairs
    NSK = S // 128       # 16 key chunks
    NPAIR = 4            # head pairs per core

    with tile.TileContext(nc) as tc:
        import contextlib

        with contextlib.ExitStack() as ctx:
            # ---------- persistent tensors + constants ----------
            persist = ctx.enter_context(tc.tile_pool(name="persist", bufs=1))
            consts = ctx.enter_context(tc.tile_pool(name="consts", bufs=1))

            qhT_sb = persist.tile([128, NPAIR, S], bf16)   # [c%128, pair, sq]
            khT_sb = persist.tile([128, NPAIR, S], bf16)
            vh_sb = persist.tile([128, NSK, 8, DK + 1], bf16)  # ones col at 64

            ones_sb = consts.tile([1, 512], bf16)
            nc.sync.dma_start(out=ones_sb, in_=ones_d[0:1, :])
            # strided scatter as a DMA would be ~16k descriptors; memset on
            # the idle gpsimd engine instead
            nc.gpsimd.memset(vh_sb[:, :, :, DK], 1.0)
            mb_sb = consts.tile([128, 16], f32)
            nc.sync.dma_start(out=mb_sb, in_=mb_d)
            wo_sb = consts.tile([128, NPAIR, D], bf16)
            for j in range(NPAIR):
                nc.sync.dma_start(
                    out=wo_sb[:, j, :], in_=wo_d[j * 128 : j * 128 + 128, :]
                )

            # ---------- phase A: transposes + projections ----------
            with contextlib.ExitStack() as actx:
                aconsts = actx.enter_context(tc.tile_pool(name="aconsts", bufs=1))
                wpool = actx.enter_context(tc.tile_pool(name="wpool", bufs=2))
                natpool = actx.enter_context(tc.tile_pool(name="natpool", bufs=6))
                xtpool = actx.enter_context(tc.tile_pool(name="xtpool", bufs=3))
                tppool = actx.enter_context(
                    tc.tile_pool(name="tppool", bufs=2, space="PSUM")
                )
                prpool = actx.enter_context(
                    tc.tile_pool(name="prpool", bufs=4, space="PSUM")
                )

                ident = aconsts.tile([128, 128], bf16)
                nc.sync.dma_start(out=ident, in_=ident_d)
                bq_sb = aconsts.tile([1, CPG], bf16)
                nc.sync.dma_start(out=bq_sb, in_=bq_d[None, :])
                bk_sb = aconsts.tile([1, CPG], bf16)
                nc.sync.dma_start(out=bk_sb, in_=bk_d[None, :])
                bv_sb = aconsts.tile([1, CPG], bf16)
                nc.sync.dma_start(out=bv_sb, in_=bv_d[None, :])

                for tname, x_d, w_d, b_sb in (
                    ("k", k_d, wk_d, bk_sb),
                    ("v", v_d, wv_d, bv_sb),
                    ("q", q_d, wq_d, bq_sb),
                ):
                    w_sb = wpool.tile([128, NDCH, CPG], bf16, tag="w")
                    first_nats = []
                    if tname == "k":
                        for i in range(4):
                            x_nat = natpool.tile([128, D], bf16, tag="nat")
                            nc.sync.dma_start(out=x_nat, in_=x_d[i * 128 : i * 128 + 128, :])
                            first_nats.append(x_nat)
                    for j in range(NDCH):
                        nc.sync.dma_start(
                            out=w_sb[:, j, :], in_=w_d[j * 128 : j * 128 + 128, :]
                        )
                    for sq in range(NSQ):
                        if sq == 0 and first_nats:
                            nats = first_nats
                        else:
                            nats = []
                            for i in range(4):
                                x_nat = natpool.tile([128, D], bf16, tag="nat")
                                r0 = sq * 512 + i * 128
                                nc.sync.dma_start(out=x_nat, in_=x_d[r0 : r0 + 128, :])
                                nats.append(x_nat)

                        # open accumulation groups: bias outer-product first
                        prs = []
                        for cch in range(4):
                            pr = prpool.tile([128, 512], f32, tag="pr")
                            prs.append(pr)
                            if tname == "v":
                                nc.tensor.matmul(
                                    pr,
                                    lhsT=ones_sb[0:1, 0:128],
                                    rhs=b_sb[0:1, :],
                                    start=True,
                                    stop=False,
                                )
                            else:
                                nc.tensor.matmul(
                                    pr,
                                    lhsT=b_sb[0:1, cch * 128 : cch * 128 + 128],
                                    rhs=ones_sb[0:1, 0:512],
                                    start=True,
                                    stop=False,
                                )

                        def emit_proj(xt_t, jp_t):
                            for jj in range(2):
                                j = jp_t * 2 + jj
                                xtj = xt_t[:, jj * 512 : jj * 512 + 512]
                                for cch in range(4):
                                    if tname == "v":
                                        # vh[sk,c]: lhsT = xT chunk, rhs = W
                                        nc.tensor.matmul(
                                            prs[cch],
                                            lhsT=xtj[:, cch * 128 : cch * 128 + 128],
                                            rhs=w_sb[:, j, :],
                                            start=False,
                                            stop=(j == NDCH - 1),
                                        )
                                    else:
                                        # qhT[c,sq]: lhsT = W chunk, rhs = xT
                                        nc.tensor.matmul(
                                            prs[cch],
                                            lhsT=w_sb[
                                                :, j, cch * 128 : cch * 128 + 128
                                            ],
                                            rhs=xtj,
                                            start=False,
                                            stop=(j == NDCH - 1),
                                        )

                        # 1-deep software pipeline: transposes for group jp+1
                        # run on the PE while the DVE copies group jp out, so
                        # projections never wait on the copy
                        pend_proj = None
                        for jp in range(NJP):
                            # one full PSUM bank holds transposes for d-chunks
                            # 2jp and 2jp+1 (bank-granular matmul writes)
                            tp_bank = tppool.tile([128, 1024], bf16, tag="tp")
                            for jj in range(2):
                                j = jp * 2 + jj
                                for i in range(4):
                                    nc.tensor.transpose(
                                        out=tp_bank[
                                            :,
                                            jj * 512 + i * 128 : jj * 512 + i * 128 + 128,
                                        ],
                                        in_=nats[i][:, j * 128 : j * 128 + 128],
                                        identity=ident,
                                    )
                            xt = xtpool.tile([128, 1024], bf16, tag="xt")
                            nc.vector.tensor_copy(out=xt, in_=tp_bank)
                            if pend_proj is not None:
                                emit_proj(*pend_proj)
                            pend_proj = (xt, jp)
                        emit_proj(*pend_proj)

                        for cch in range(4):
                            if tname == "v":
                                skc = sq * 4 + cch
                                nc.vector.tensor_copy(
                                    out=vh_sb[:, skc, :, 0:DK],
                                    in_=prs[cch].rearrange("p (h d) -> p h d", h=8),
                                )
                            else:
                                dst = qhT_sb if tname == "q" else khT_sb
                                nc.vector.tensor_copy(
                                    out=dst[:, cch, sq * 512 : sq * 512 + 512],
                                    in_=prs[cch],
                                )

            # ---------- phase B: attention ----------
            concpool = ctx.enter_context(tc.tile_pool(name="concpool", bufs=1))
            concatT_sb = concpool.tile([128, NPAIR, S], bf16)
            with contextlib.ExitStack() as bctx:
                probpool = bctx.enter_context(tc.tile_pool(name="probpool", bufs=3))
                rppool = bctx.enter_context(tc.tile_pool(name="rppool", bufs=2))
                copool = bctx.enter_context(tc.tile_pool(name="copool", bufs=3))
                scpool = bctx.enter_context(
                    tc.tile_pool(name="scpool", bufs=2, space="PSUM")
                )
                atpool = bctx.enter_context(
                    tc.tile_pool(name="atpool", bufs=2, space="PSUM")
                )

                chunks = [
                    (sq2, pair, hh)
                    for sq2 in range(S // 1024)
                    for pair in range(NPAIR)
                    for hh in range(2)
                ]

                def emit_norm(state):
                    """Normalization tail for a finished head-chunk: denom ->
                    1/denom (p0), broadcast via K=1 matmul into a borrowed sc
                    bank, multiply staged attn rows into concatT."""
                    (sq2, pair, hh), at_ps = state
                    base = hh * 64
                    # custom-DVE recip needs partition-0 input: native copy
                    # shifts the denominator row 64 -> 0
                    den0 = rppool.tile([1, 1024], f32, tag="den0")
                    nc.vector.tensor_copy(out=den0, in_=at_ps[64:65, :])
                    recip32 = rppool.tile([1, 1024], f32, tag="recip32")
                    nc.vector.reciprocal_approx_fast(out=recip32, in_=den0)
                    recip = rppool.tile([1, 1024], bf16, tag="recip")
                    nc.vector.tensor_copy(out=recip, in_=recip32)
                    # DVE can read only one PSUM operand per op: stage attn
                    attn_sb = rppool.tile([64, 1024], f32, tag="attn")
                    nc.vector.tensor_copy(out=attn_sb, in_=at_ps[0:64, :])
                    for half in range(2):
                        rep_bank = scpool.tile([128, 1024], f32, tag="sc")
                        rep_ps = rep_bank[0:64, 0:512]
                        nc.tensor.matmul(
                            rep_ps,
                            lhsT=ones_sb[0:1, 0:64],
                            rhs=recip[0:1, half * 512 : half * 512 + 512],
                            start=True,
                            stop=True,
                        )
                        nc.vector.tensor_mul(
                            concatT_sb[
                                base : base + 64,
                                pair,
                                sq2 * 1024 + half * 512 : sq2 * 1024
                                + half * 512
                                + 512,
                            ],
                            attn_sb[:, half * 512 : half * 512 + 512],
                            rep_ps,
                        )

                def emit_cgroup(sqc, do):
                    """Out-projection group for 128 query rows x 512 out cols,
                    into a borrowed sc bank; fills PE gaps in the ACT-bound
                    attention loop (also keeps the PE p-state high)."""
                    cb = scpool.tile([128, 1024], f32, tag="sc")
                    o_ps = cb[:, 0:512]
                    for j in range(NPAIR):
                        nc.tensor.matmul(
                            o_ps,
                            lhsT=concatT_sb[:, j, sqc * 128 : sqc * 128 + 128],
                            rhs=wo_sb[:, j, do * 512 : do * 512 + 512],
                            start=(j == 0),
                            stop=(j == NPAIR - 1),
                        )
                    o_sb = copool.tile([128, 512], f32, tag="osb")
                    nc.vector.tensor_copy(out=o_sb, in_=o_ps)
                    nc.sync.dma_start(
                        out=out_d[
                            sqc * 128 : sqc * 128 + 128,
                            do * 512 : do * 512 + 512,
                        ],
                        in_=o_sb,
                    )

                cqueue = []
                pending = None
                for ci, (sq2, pair, hh) in enumerate(chunks):
                    h = pair * 2 + hh
                    base = hh * 64
                    at_ps = atpool.tile([128, 1024], f32, tag="at")
                    prev_probs = None
                    for sk in range(NSK):
                        sc_ps = scpool.tile([128, 1024], f32, tag="sc")
                        for half in range(2):
                            qoff = sq2 * 1024 + half * 512
                            nc.tensor.matmul(
                                sc_ps[:, half * 512 : half * 512 + 512],
                                lhsT=khT_sb[
                                    base : base + 64,
                                    pair,
                                    sk * 128 : sk * 128 + 128,
                                ],
                                rhs=qhT_sb[
                                    base : base + 64, pair, qoff : qoff + 512
                                ],
                                start=True,
                                stop=True,
                            )
                        probs = probpool.tile([128, 1024], bf16, tag="probs")
                        nc.scalar.activation(
                            out=probs,
                            in_=sc_ps,
                            func=Exp,
                            bias=mb_sb[:, sk : sk + 1],
                            scale=0.125,
                        )
                        if prev_probs is not None:
                            psk, pp = prev_probs
                            for half in range(2):
                                nc.tensor.matmul(
                                    at_ps[0:65, half * 512 : half * 512 + 512],
                                    lhsT=vh_sb[:, psk, h, :],
                                    rhs=pp[:, half * 512 : half * 512 + 512],
                                    start=(psk == 0),
                                    stop=False,
                                )
                        prev_probs = (sk, probs)
                        if sk == 2 and pending is not None:
                            emit_norm(pending)
                            pending = None
                        if cqueue and ci >= 9 and sk in (6, 10, 13):
                            emit_cgroup(*cqueue.pop(0))
                    psk, pp = prev_probs
                    for half in range(2):
                        nc.tensor.matmul(
                            at_ps[0:65, half * 512 : half * 512 + 512],
                            lhsT=vh_sb[:, psk, h, :],
                            rhs=pp[:, half * 512 : half * 512 + 512],
                            start=False,
                            stop=True,
                        )
                    pending = ((sq2, pair, hh), at_ps)
                    if pair == NPAIR - 1 and hh == 1:
                        # this sq2 block's 1024 query rows are (almost) done;
                        # queue their out-projection groups
                        for sqc in range(sq2 * 8, sq2 * 8 + 8):
                            for do in range(2):
                                cqueue.append((sqc, do))
                emit_norm(pending)
                for g in cqueue:
                    emit_cgroup(*g)

    nc.compile()
    return nc


def get_nc():
    if "nc" not in _cache:
        _cache["nc"] = _build_nc()
    return _cache["nc"]


def make_in_maps(q, k, v, mask, Wq, bq, Wk, bk, Wv, bv, Wo, bo):
    import ml_dtypes

    f32 = np.float32
    bf16 = ml_dtypes.bfloat16
    c = np.ascontiguousarray
    in_maps = []
    for core in range(NCORES):
        b, g = core // 2, core % 2
        cols = slice(g * CPG, (g + 1) * CPG)
        mb = (-1e9 * (1.0 - np.asarray(mask[b, 0], f32))).reshape(16, 128).T
        in_maps.append(
            {
                "q": c(np.asarray(q[b], f32).astype(bf16)),
                "k": c(np.asarray(k[b], f32).astype(bf16)),
                "v": c(np.asarray(v[b], f32).astype(bf16)),
                "wq": c(np.asarray(Wq[:, cols], f32).astype(bf16)),
                "wk": c(np.asarray(Wk[:, cols], f32).astype(bf16)),
                "wv": c(np.asarray(Wv[:, cols], f32).astype(bf16)),
                "wo": c(np.asarray(Wo[cols, :], f32).astype(bf16)),
                "bq": c(np.asarray(bq[cols], f32).astype(bf16)),
                "bk": c(np.asarray(bk[cols], f32).astype(bf16)),
                "bv": c(np.asarray(bv[cols], f32).astype(bf16)),
                "maskbias": c(mb),
                "ones": np.ones((128, 512), bf16),
                "ident": np.eye(128, dtype=bf16),
            }
        )
    return in_maps


def gather(results, bo):
    out = np.zeros((B, S, D), np.float32)
    for core in range(NCORES):
        b = core // 2
        out[b] += results[core]["out"]
    out += np.asarray(bo, np.float32)[None, None, :]
    return out


def run_on_hw(in_maps, trace=False, trace_cores=None):
    from concourse.bass_utils import run_bass_kernel_spmd

    nc = get_nc()
    return run_bass_kernel_spmd(
        nc,
        in_maps,
        list(range(NCORES)),
        trace=trace,
        trace_cores=trace_cores,
    )


def kernel(q, k, v, mask, Wq, bq, Wk, bk, Wv, bv, Wo, bo):
    in_maps = make_in_maps(q, k, v, mask, Wq, bq, Wk, bk, Wv, bv, Wo, bo)
    res = run_on_hw(in_maps)
    return gather(res.results, bo)


# revision 20
# speedup vs baseline: 1.7206x; 1.2872x over previous
"""Multi-head attention (B=4, S=2048, D=1024, H=16, Dk=64) on 8 trn2 NeuronCores.

Sharding: core = (batch b, head-group g) with b in 0..3, g in 0..1.
Each core computes attention for its batch and its 8 heads, plus the partial
out-projection for its 512 columns of Wo.  Host sums the two partials per
batch and adds bo.

Per-core kernel (matmuls bf16, f32 PSUM accumulation):
  phase A: PE-transpose q/k/v 128x128 tiles (bf16, 2 d-chunks per PSUM bank)
           and project into f32 PSUM; qhT/khT stored bf16 [c=512, s=2048]
           (c on partitions, head pairs per 128-partition tile), vh stored
           bf16 [s, c] with a ones column per head ([128, 16, 8, 65]).
           Biases fold in as K=1 outer-product matmuls opening each group.
  phase B (per 1024-wide query chunk, per head), software-pipelined so the
           PE always has a score matmul to run while ACT computes exp:
           scoresT[sk,sq] = khT_h^T @ qhT_h  (K=64)
           probsT = exp(scoresT/8 + maskbias[sk]) -> bf16 (mask folds into
           the ACT bias; masked keys underflow to exactly 0)
           attnT[c(+sum),sq] += vh^T @ probsT (ones column accumulates the
           softmax denominator in row 64)
           normalize: denom row -> partition 0, fast approx reciprocal,
           broadcast with a K=1 matmul into a borrowed sc bank, multiply.
  phase C: out[sq,:] = concatT^T @ Wo  (accumulate 4 c-chunks in PSUM)
"""

import os
import sys

sys.path.insert(0, "/opt/trn_rl_repo")

import numpy as np

B, S, D, H, DK = 4, 2048, 1024, 16, 64
CPG = 512          # projection columns per core (8 heads x 64)
NCORES = 8

_cache = {}


def _build_nc():
    import concourse.bass as bass
    import concourse.tile as tile
    from concourse import bacc, mybir

    f32 = mybir.dt.float32
    bf16 = mybir.dt.bfloat16
    Exp = mybir.ActivationFunctionType.Exp

    nc = bacc.Bacc("TRN2", target_bir_lowering=False, debug=False)

    q_d = nc.dram_tensor("q", [S, D], bf16, kind="ExternalInput").ap()
    k_d = nc.dram_tensor("k", [S, D], bf16, kind="ExternalInput").ap()
    v_d = nc.dram_tensor("v", [S, D], bf16, kind="ExternalInput").ap()
    wq_d = nc.dram_tensor("wq", [D, CPG], bf16, kind="ExternalInput").ap()
    wk_d = nc.dram_tensor("wk", [D, CPG], bf16, kind="ExternalInput").ap()
    wv_d = nc.dram_tensor("wv", [D, CPG], bf16, kind="ExternalInput").ap()
    wo_d = nc.dram_tensor("wo", [CPG, D], bf16, kind="ExternalInput").ap()
    bq_d = nc.dram_tensor("bq", [CPG], bf16, kind="ExternalInput").ap()
    bk_d = nc.dram_tensor("bk", [CPG], bf16, kind="ExternalInput").ap()
    bv_d = nc.dram_tensor("bv", [CPG], bf16, kind="ExternalInput").ap()
    mb_d = nc.dram_tensor("maskbias", [128, 16], f32, kind="ExternalInput").ap()
    ones_d = nc.dram_tensor("ones", [128, 512], bf16, kind="ExternalInput").ap()
    ident_d = nc.dram_tensor("ident", [128, 128], bf16, kind="ExternalInput").ap()
    out_d = nc.dram_tensor("out", [S, D], f32, kind="ExternalOutput").ap()

    NSQ = S // 512       # 4 query/key 512-blocks
    NDCH = D // 128      # 8 contraction chunks for projections
    NJP = NDCH // 2      # transpose bank pairs
    NSK = S // 128       # 16 key chunks
    NPAIR = 4            # head pairs per core

    with tile.TileContext(nc) as tc:
        import contextlib

        with contextlib.ExitStack() as ctx:
            # ---------- persistent tensors + constants ----------
            persist = ctx.enter_context(tc.tile_pool(name="persist", bufs=1))
            consts = ctx.enter_context(tc.tile_pool(name="consts", bufs=1))

            qhT_sb = persist.tile([128, NPAIR, S], bf16)   # [c%128, pair, sq]
            khT_sb = persist.tile([128, NPAIR, S], bf16)
            vh_sb = persist.tile([128, NSK, 8, DK + 1], bf16)  # ones col at 64

            ones_sb = consts.tile([1, 512], bf16)
            nc.sync.dma_start(out=ones_sb, in_=ones_d[0:1, :])
            # strided scatter as a DMA would be ~16k descriptors; memset on
            # the idle gpsimd engine instead
            nc.gpsimd.memset(vh_sb[:, :, :, DK], 1.0)
            mb_sb = consts.tile([128, 16], f32)
            nc.sync.dma_start(out=mb_sb, in_=mb_d)
            wo_sb = consts.tile([128, NPAIR, D], bf16)
            for j in range(NPAIR):
                nc.sync.dma_start(
                    out=wo_sb[:, j, :], in_=wo_d[j * 128 : j * 128 + 128, :]
                )

            # ---------- phase A: transposes + projections ----------
            with contextlib.ExitStack() as actx:
                aconsts = actx.enter_context(tc.tile_pool(name="aconsts", bufs=1))
                wpool = actx.enter_context(tc.tile_pool(name="wpool", bufs=2))
                natpool = actx.enter_context(tc.tile_pool(name="natpool", bufs=6))
                xtpool = actx.enter_context(tc.tile_pool(name="xtpool", bufs=3))
                tppool = actx.enter_context(
                    tc.tile_pool(name="tppool", bufs=2, space="PSUM")
                )
                prpool = actx.enter_context(
                    tc.tile_pool(name="prpool", bufs=4, space="PSUM")
                )

                ident = aconsts.tile([128, 128], bf16)
                nc.sync.dma_start(out=ident, in_=ident_d)
                bq_sb = aconsts.tile([1, CPG], bf16)
                nc.sync.dma_start(out=bq_sb, in_=bq_d[None, :])
                bk_sb = aconsts.tile([1, CPG], bf16)
                nc.sync.dma_start(out=bk_sb, in_=bk_d[None, :])
                bv_sb = aconsts.tile([1, CPG], bf16)
                nc.sync.dma_start(out=bv_sb, in_=bv_d[None, :])

                for tname, x_d, w_d, b_sb in (
                    ("k", k_d, wk_d, bk_sb),
                    ("v", v_d, wv_d, bv_sb),
                    ("q", q_d, wq_d, bq_sb),
                ):
                    w_sb = wpool.tile([128, NDCH, CPG], bf16, tag="w")
                    first_nats = []
                    if tname == "k":
                        for i in range(4):
                            x_nat = natpool.tile([128, D], bf16, tag="nat")
                            nc.sync.dma_start(out=x_nat, in_=x_d[i * 128 : i * 128 + 128, :])
                            first_nats.append(x_nat)
                    for j in range(NDCH):
                        nc.sync.dma_start(
                            out=w_sb[:, j, :], in_=w_d[j * 128 : j * 128 + 128, :]
                        )
                    for sq in range(NSQ):
                        if sq == 0 and first_nats:
                            nats = first_nats
                        else:
                            nats = []
                            for i in range(4):
                                x_nat = natpool.tile([128, D], bf16, tag="nat")
                                r0 = sq * 512 + i * 128
                                nc.sync.dma_start(out=x_nat, in_=x_d[r0 : r0 + 128, :])
                                nats.append(x_nat)

                        # open accumulation groups: bias outer-product first
                        prs = []
                        for cch in range(4):
                            pr = prpool.tile([128, 512], f32, tag="pr")
                            prs.append(pr)
                            if tname == "v":
                                nc.tensor.matmul(
                                    pr,
                                    lhsT=ones_sb[0:1, 0:128],
                                    rhs=b_sb[0:1, :],
                                    start=True,
                                    stop=False,
                                )
                            else:
                                nc.tensor.matmul(
                                    pr,
                                    lhsT=b_sb[0:1, cch * 128 : cch * 128 + 128],
                                    rhs=ones_sb[0:1, 0:512],
                                    start=True,
                                    stop=False,
                                )

                        def emit_proj(xt_t, jp_t):
                            for jj in range(2):
                                j = jp_t * 2 + jj
                                xtj = xt_t[:, jj * 512 : jj * 512 + 512]
                                for cch in range(4):
                                    if tname == "v":
                                        # vh[sk,c]: lhsT = xT chunk, rhs = W
                                        nc.tensor.matmul(
                                            prs[cch],
                                            lhsT=xtj[:, cch * 128 : cch * 128 + 128],
                                            rhs=w_sb[:, j, :],
                                            start=False,
                                            stop=(j == NDCH - 1),
                                        )
                                    else:
                                        # qhT[c,sq]: lhsT = W chunk, rhs = xT
                                        nc.tensor.matmul(
                                            prs[cch],
                                            lhsT=w_sb[
                                                :, j, cch * 128 : cch * 128 + 128
                                            ],
                                            rhs=xtj,
                                            start=False,
                                            stop=(j == NDCH - 1),
                                        )

                        # 1-deep software pipeline: transposes for group jp+1
                        # run on the PE while the DVE copies group jp out, so
                        # projections never wait on the copy
                        pend_proj = None
                        for jp in range(NJP):
                            # one full PSUM bank holds transposes for d-chunks
                            # 2jp and 2jp+1 (bank-granular matmul writes)
                            tp_bank = tppool.tile([128, 1024], bf16, tag="tp")
                            for jj in range(2):
                                j = jp * 2 + jj
                                for i in range(4):
                                    nc.tensor.transpose(
                                        out=tp_bank[
                                            :,
                                            jj * 512 + i * 128 : jj * 512 + i * 128 + 128,
                                        ],
                                        in_=nats[i][:, j * 128 : j * 128 + 128],
                                        identity=ident,
                                    )
                            xt = xtpool.tile([128, 1024], bf16, tag="xt")
                            nc.vector.tensor_copy(out=xt, in_=tp_bank)
                            if pend_proj is not None:
                                emit_proj(*pend_proj)
                            pend_proj = (xt, jp)
                        emit_proj(*pend_proj)

                        for cch in range(4):
                            if tname == "v":
                                skc = sq * 4 + cch
                                nc.vector.tensor_copy(
                                    out=vh_sb[:, skc, :, 0:DK],
                                    in_=prs[cch].rearrange("p (h d) -> p h d", h=8),
                                )
                            else:
                                dst = qhT_sb if tname == "q" else khT_sb
                                nc.vector.tensor_copy(
                                    out=dst[:, cch, sq * 512 : sq * 512 + 512],
                                    in_=prs[cch],
                                )

            # ---------- phase B: attention ----------
            concpool = ctx.enter_context(tc.tile_pool(name="concpool", bufs=1))
            concatT_sb = concpool.tile([128, NPAIR, S], bf16)
            with contextlib.ExitStack() as bctx:
                probpool = bctx.enter_context(tc.tile_pool(name="probpool", bufs=4))
                rppool = bctx.enter_context(tc.tile_pool(name="rppool", bufs=2))
                copool = bctx.enter_context(tc.tile_pool(name="copool", bufs=3))
                scpool = bctx.enter_context(
                    tc.tile_pool(name="scpool", bufs=2, space="PSUM")
                )
                atpool = bctx.enter_context(
                    tc.tile_pool(name="atpool", bufs=2, space="PSUM")
                )

                chunks = [
                    (sq2, pair, hh)
                    for sq2 in range(S // 1024)
                    for pair in range(NPAIR)
                    for hh in range(2)
                ]

                def emit_norm_head(state):
                    """DVE part of a finished head-chunk's normalization:
                    denom -> 1/denom at partition 0, stage attn in SBUF."""
                    (sq2, pair, hh), at_ps = state
                    # custom-DVE recip needs partition-0 input: native copy
                    # shifts the denominator row 64 -> 0
                    den0 = rppool.tile([1, 1024], f32, tag="den0")
                    nc.vector.tensor_copy(out=den0, in_=at_ps[64:65, :])
                    recip32 = rppool.tile([1, 1024], f32, tag="recip32")
                    nc.vector.reciprocal_approx_fast(out=recip32, in_=den0)
                    recip = rppool.tile([1, 1024], bf16, tag="recip")
                    nc.vector.tensor_copy(out=recip, in_=recip32)
                    # DVE can read only one PSUM operand per op: stage attn
                    attn_sb = rppool.tile([64, 1024], f32, tag="attn")
                    nc.vector.tensor_copy(out=attn_sb, in_=at_ps[0:64, :])
                    return ((sq2, pair, hh), recip, attn_sb)

                def emit_norm_tail(state):
                    """PE/DVE finish emitted several periods later so the rep
                    matmul never stalls the PE on the recip chain."""
                    (sq2, pair, hh), recip, attn_sb = state
                    base = hh * 64
                    for half in range(2):
                        rep_bank = scpool.tile([128, 1024], f32, tag="sc")
                        rep_ps = rep_bank[0:64, 0:512]
                        nc.tensor.matmul(
                            rep_ps,
                            lhsT=ones_sb[0:1, 0:64],
                            rhs=recip[0:1, half * 512 : half * 512 + 512],
                            start=True,
                            stop=True,
                        )
                        nc.vector.tensor_mul(
                            concatT_sb[
                                base : base + 64,
                                pair,
                                sq2 * 1024 + half * 512 : sq2 * 1024
                                + half * 512
                                + 512,
                            ],
                            attn_sb[:, half * 512 : half * 512 + 512],
                            rep_ps,
                        )

                def emit_cgroup(sqc, do):
                    """Out-projection group for 128 query rows x 512 out cols,
                    into a borrowed sc bank; fills PE gaps in the ACT-bound
                    attention loop (also keeps the PE p-state high)."""
                    cb = scpool.tile([128, 1024], f32, tag="sc")
                    o_ps = cb[:, 0:512]
                    for j in range(NPAIR):
                        nc.tensor.matmul(
                            o_ps,
                            lhsT=concatT_sb[:, j, sqc * 128 : sqc * 128 + 128],
                            rhs=wo_sb[:, j, do * 512 : do * 512 + 512],
                            start=(j == 0),
                            stop=(j == NPAIR - 1),
                        )
                    o_sb = copool.tile([128, 512], f32, tag="osb")
                    nc.vector.tensor_copy(out=o_sb, in_=o_ps)
                    nc.sync.dma_start(
                        out=out_d[
                            sqc * 128 : sqc * 128 + 128,
                            do * 512 : do * 512 + 512,
                        ],
                        in_=o_sb,
                    )

                cqueue = []
                pending = None      # finished chunk awaiting norm head
                pending_tail = None  # norm head awaiting rep/mul finish
                for ci, (sq2, pair, hh) in enumerate(chunks):
                    h = pair * 2 + hh
                    base = hh * 64
                    at_ps = atpool.tile([128, 1024], f32, tag="at")
                    probs_q = []  # 2-deep: attn runs on probs from sk-2
                    for sk in range(NSK):
                        sc_ps = scpool.tile([128, 1024], f32, tag="sc")
                        for half in range(2):
                            qoff = sq2 * 1024 + half * 512
                            nc.tensor.matmul(
                                sc_ps[:, half * 512 : half * 512 + 512],
                                lhsT=khT_sb[
                                    base : base + 64,
                                    pair,
                                    sk * 128 : sk * 128 + 128,
                                ],
                                rhs=qhT_sb[
                                    base : base + 64, pair, qoff : qoff + 512
                                ],
                                start=True,
                                stop=True,
                            )
                        probs = probpool.tile([128, 1024], bf16, tag="probs")
                        nc.scalar.activation(
                            out=probs,
                            in_=sc_ps,
                            func=Exp,
                            bias=mb_sb[:, sk : sk + 1],
                            scale=0.125,
                        )
                        probs_q.append((sk, probs))
                        if len(probs_q) > 2:
                            psk, pp = probs_q.pop(0)
                            for half in range(2):
                                nc.tensor.matmul(
                                    at_ps[0:65, half * 512 : half * 512 + 512],
                                    lhsT=vh_sb[:, psk, h, :],
                                    rhs=pp[:, half * 512 : half * 512 + 512],
                                    start=(psk == 0),
                                    stop=False,
                                )
                        if sk == 2 and pending is not None:
                            pending_tail = emit_norm_head(pending)
                            pending = None
                        if sk == 6 and pending_tail is not None:
                            emit_norm_tail(pending_tail)
                            pending_tail = None
                        if cqueue and ci >= 9 and sk in (9, 12, 14):
                            emit_cgroup(*cqueue.pop(0))
                    for psk, pp in probs_q:
                        for half in range(2):
                            nc.tensor.matmul(
                                at_ps[0:65, half * 512 : half * 512 + 512],
                                lhsT=vh_sb[:, psk, h, :],
                                rhs=pp[:, half * 512 : half * 512 + 512],
                                start=(psk == 0),
                                stop=(psk == NSK - 1),
                            )
                    pending = ((sq2, pair, hh), at_ps)
                    if pair == NPAIR - 1 and hh == 1:
                        # this sq2 block's 1024 query rows are (almost) done;
                        # queue their out-projection groups
                        for sqc in range(sq2 * 8, sq2 * 8 + 8):
                            for do in range(2):
                                cqueue.append((sqc, do))
                pending_tail2 = emit_norm_head(pending)
                emit_norm_tail(pending_tail2)
                for g in cqueue:
                    emit_cgroup(*g)

    nc.compile()
    return nc


def get_nc():
    if "nc" not in _cache:
        _cache["nc"] = _build_nc()
    return _cache["nc"]


def make_in_maps(q, k, v, mask, Wq, bq, Wk, bk, Wv, bv, Wo, bo):
    import ml_dtypes

    f32 = np.float32
    bf16 = ml_dtypes.bfloat16
    c = np.ascontiguousarray
    in_maps = []
    for core in range(NCORES):
        b, g = core // 2, core % 2
        cols = slice(g * CPG, (g + 1) * CPG)
        mb = (-1e9 * (1.0 - np.asarray(mask[b, 0], f32))).reshape(16, 128).T
        in_maps.append(
            {
                "q": c(np.asarray(q[b], f32).astype(bf16)),
                "k": c(np.asarray(k[b], f32).astype(bf16)),
                "v": c(np.asarray(v[b], f32).astype(bf16)),
                "wq": c(np.asarray(Wq[:, cols], f32).astype(bf16)),
                "wk": c(np.asarray(Wk[:, cols], f32).astype(bf16)),
                "wv": c(np.asarray(Wv[:, cols], f32).astype(bf16)),
                "wo": c(np.asarray(Wo[cols, :], f32).astype(bf16)),
                "bq": c(np.asarray(bq[cols], f32).astype(bf16)),
                "bk": c(np.asarray(bk[cols], f32).astype(bf16)),
                "bv": c(np.asarray(bv[cols], f32).astype(bf16)),
                "maskbias": c(mb),
                "ones": np.ones((128, 512), bf16),
                "ident": np.eye(128, dtype=bf16),
            }
        )
    return in_maps


def gather(results, bo):
    out = np.zeros((B, S, D), np.float32)
    for core in range(NCORES):
        b = core // 2
        out[b] += results[core]["out"]
    out += np.asarray(bo, np.float32)[None, None, :]
    return out


def run_on_hw(in_maps, trace=False, trace_cores=None):
    from concourse.bass_utils import run_bass_kernel_spmd

    nc = get_nc()
    return run_bass_kernel_spmd(
        nc,
        in_maps,
        list(range(NCORES)),
        trace=trace,
        trace_cores=trace_cores,
    )


def kernel(q, k, v, mask, Wq, bq, Wk, bk, Wv, bv, Wo, bo):
    in_maps = make_in_maps(q, k, v, mask, Wq, bq, Wk, bk, Wv, bv, Wo, bo)
    res = run_on_hw(in_maps)
    return gather(res.results, bo)


# revision 25
# speedup vs baseline: 1.7516x; 1.0180x over previous
"""Multi-head attention (B=4, S=2048, D=1024, H=16, Dk=64) on 8 trn2 NeuronCores.

Sharding: core = (batch b, head-group g) with b in 0..3, g in 0..1.
Each core computes attention for its batch and its 8 heads, plus the partial
out-projection for its 512 columns of Wo.  Host sums the two partials per
batch and adds bo.

Per-core kernel (matmuls bf16, f32 PSUM accumulation):
  phase A: PE-transpose q/k/v 128x128 tiles (bf16, 2 d-chunks per PSUM bank)
           and project into f32 PSUM; qhT/khT stored bf16 [c=512, s=2048]
           (c on partitions, head pairs per 128-partition tile), vh stored
           bf16 [s, c] with a ones column per head ([128, 16, 8, 65]).
           Biases fold in as K=1 outer-product matmuls opening each group.
  phase B (per 1024-wide query chunk, per head), software-pipelined so the
           PE always has a score matmul to run while ACT computes exp:
           scoresT[sk,sq] = khT_h^T @ qhT_h  (K=64)
           probsT = exp(scoresT/8 + maskbias[sk]) -> bf16 (mask folds into
           the ACT bias; masked keys underflow to exactly 0)
           attnT[c(+sum),sq] += vh^T @ probsT (ones column accumulates the
           softmax denominator in row 64)
           normalize: denom row -> partition 0, fast approx reciprocal,
           broadcast with a K=1 matmul into a borrowed sc bank, multiply.
  phase C: out[sq,:] = concatT^T @ Wo  (accumulate 4 c-chunks in PSUM)
"""

import os
import sys

sys.path.insert(0, "/opt/trn_rl_repo")

import numpy as np

B, S, D, H, DK = 4, 2048, 1024, 16, 64
CPG = 512          # projection columns per core (8 heads x 64)
NCORES = 8

_cache = {}


def _build_nc():
    import concourse.bass as bass
    import concourse.tile as tile
    from concourse import bacc, mybir

    f32 = mybir.dt.float32
    bf16 = mybir.dt.bfloat16
    Exp = mybir.ActivationFunctionType.Exp

    nc = bacc.Bacc("TRN2", target_bir_lowering=False, debug=False)

    q_d = nc.dram_tensor("q", [S, D], bf16, kind="ExternalInput").ap()
    k_d = nc.dram_tensor("k", [S, D], bf16, kind="ExternalInput").ap()
    v_d = nc.dram_tensor("v", [S, D], bf16, kind="ExternalInput").ap()
    wq_d = nc.dram_tensor("wq", [D, CPG], bf16, kind="ExternalInput").ap()
    wk_d = nc.dram_tensor("wk", [D, CPG], bf16, kind="ExternalInput").ap()
    wv_d = nc.dram_tensor("wv", [D, CPG], bf16, kind="ExternalInput").ap()
    wo_d = nc.dram_tensor("wo", [CPG, D], bf16, kind="ExternalInput").ap()
    bq_d = nc.dram_tensor("bq", [CPG], bf16, kind="ExternalInput").ap()
    bk_d = nc.dram_tensor("bk", [CPG], bf16, kind="ExternalInput").ap()
    bv_d = nc.dram_tensor("bv", [CPG], bf16, kind="ExternalInput").ap()
    mb_d = nc.dram_tensor("maskbias", [128, 16], f32, kind="ExternalInput").ap()
    ones_d = nc.dram_tensor("ones", [128, 512], bf16, kind="ExternalInput").ap()
    ident_d = nc.dram_tensor("ident", [128, 128], bf16, kind="ExternalInput").ap()
    out_d = nc.dram_tensor("out", [S, D], f32, kind="ExternalOutput").ap()

    NSQ = S // 512       # 4 query/key 512-blocks
    NDCH = D // 128      # 8 contraction chunks for projections
    NJP = NDCH // 2      # transpose bank pairs
    NSK = S // 128       # 16 key chunks
    NPAIR = 4            # head pairs per core

    with tile.TileContext(nc) as tc:
        import contextlib

        with contextlib.ExitStack() as ctx:
            # ---------- persistent tensors + constants ----------
            persist = ctx.enter_context(tc.tile_pool(name="persist", bufs=1))
            consts = ctx.enter_context(tc.tile_pool(name="consts", bufs=1))

            qhT_sb = persist.tile([128, NPAIR, S], bf16)   # [c%128, pair, sq]
            khT_sb = persist.tile([128, NPAIR, S], bf16)
            vh_sb = persist.tile([128, NSK, 8, DK + 1], bf16)  # ones col at 64

            ones_sb = consts.tile([1, 512], bf16)
            nc.sync.dma_start(out=ones_sb, in_=ones_d[0:1, :])
            # strided scatter as a DMA would be ~16k descriptors; memset on
            # the idle gpsimd engine instead
            nc.gpsimd.memset(vh_sb[:, :, :, DK], 1.0)
            mb_sb = consts.tile([128, 16], f32)
            nc.sync.dma_start(out=mb_sb, in_=mb_d)
            wo_sb = consts.tile([128, NPAIR, D], bf16)
            for j in range(NPAIR):
                nc.sync.dma_start(
                    out=wo_sb[:, j, :], in_=wo_d[j * 128 : j * 128 + 128, :]
                )

            # ---------- phase A: transposes + projections ----------
            with contextlib.ExitStack() as actx:
                aconsts = actx.enter_context(tc.tile_pool(name="aconsts", bufs=1))
                wpool = actx.enter_context(tc.tile_pool(name="wpool", bufs=2))
                natpool = actx.enter_context(tc.tile_pool(name="natpool", bufs=6))
                xtpool = actx.enter_context(tc.tile_pool(name="xtpool", bufs=3))
                tppool = actx.enter_context(
                    tc.tile_pool(name="tppool", bufs=2, space="PSUM")
                )
                prpool = actx.enter_context(
                    tc.tile_pool(name="prpool", bufs=4, space="PSUM")
                )

                ident = aconsts.tile([128, 128], bf16)
                nc.sync.dma_start(out=ident, in_=ident_d)
                bq_sb = aconsts.tile([1, CPG], bf16)
                nc.sync.dma_start(out=bq_sb, in_=bq_d[None, :])
                bk_sb = aconsts.tile([1, CPG], bf16)
                nc.sync.dma_start(out=bk_sb, in_=bk_d[None, :])
                bv_sb = aconsts.tile([1, CPG], bf16)
                nc.sync.dma_start(out=bv_sb, in_=bv_d[None, :])

                for tname, x_d, w_d, b_sb in (
                    ("k", k_d, wk_d, bk_sb),
                    ("v", v_d, wv_d, bv_sb),
                    ("q", q_d, wq_d, bq_sb),
                ):
                    w_sb = wpool.tile([128, NDCH, CPG], bf16, tag="w")
                    first_nats = []
                    if tname == "k":
                        for i in range(4):
                            x_nat = natpool.tile([128, D], bf16, tag="nat")
                            nc.sync.dma_start(out=x_nat, in_=x_d[i * 128 : i * 128 + 128, :])
                            first_nats.append(x_nat)
                    for j in range(NDCH):
                        nc.sync.dma_start(
                            out=w_sb[:, j, :], in_=w_d[j * 128 : j * 128 + 128, :]
                        )
                    for sq in range(NSQ):
                        if sq == 0 and first_nats:
                            nats = first_nats
                        else:
                            nats = []
                            for i in range(4):
                                x_nat = natpool.tile([128, D], bf16, tag="nat")
                                r0 = sq * 512 + i * 128
                                nc.sync.dma_start(out=x_nat, in_=x_d[r0 : r0 + 128, :])
                                nats.append(x_nat)

                        # open accumulation groups: bias outer-product first
                        prs = []
                        for cch in range(4):
                            pr = prpool.tile([128, 512], f32, tag="pr")
                            prs.append(pr)
                            if tname == "v":
                                nc.tensor.matmul(
                                    pr,
                                    lhsT=ones_sb[0:1, 0:128],
                                    rhs=b_sb[0:1, :],
                                    start=True,
                                    stop=False,
                                )
                            else:
                                nc.tensor.matmul(
                                    pr,
                                    lhsT=b_sb[0:1, cch * 128 : cch * 128 + 128],
                                    rhs=ones_sb[0:1, 0:512],
                                    start=True,
                                    stop=False,
                                )

                        def emit_proj(xt_t, jp_t):
                            for jj in range(2):
                                j = jp_t * 2 + jj
                                xtj = xt_t[:, jj * 512 : jj * 512 + 512]
                                for cch in range(4):
                                    if tname == "v":
                                        # vh[sk,c]: lhsT = xT chunk, rhs = W
                                        nc.tensor.matmul(
                                            prs[cch],
                                            lhsT=xtj[:, cch * 128 : cch * 128 + 128],
                                            rhs=w_sb[:, j, :],
                                            start=False,
                                            stop=(j == NDCH - 1),
                                        )
                                    else:
                                        # qhT[c,sq]: lhsT = W chunk, rhs = xT
                                        nc.tensor.matmul(
                                            prs[cch],
                                            lhsT=w_sb[
                                                :, j, cch * 128 : cch * 128 + 128
                                            ],
                                            rhs=xtj,
                                            start=False,
                                            stop=(j == NDCH - 1),
                                        )

                        # 1-deep software pipeline: transposes for group jp+1
                        # run on the PE while the DVE copies group jp out, so
                        # projections never wait on the copy
                        pend_proj = None
                        for jp in range(NJP):
                            # one full PSUM bank holds transposes for d-chunks
                            # 2jp and 2jp+1 (bank-granular matmul writes)
                            tp_bank = tppool.tile([128, 1024], bf16, tag="tp")
                            for jj in range(2):
                                j = jp * 2 + jj
                                for i in range(4):
                                    nc.tensor.transpose(
                                        out=tp_bank[
                                            :,
                                            jj * 512 + i * 128 : jj * 512 + i * 128 + 128,
                                        ],
                                        in_=nats[i][:, j * 128 : j * 128 + 128],
                                        identity=ident,
                                    )
                            xt = xtpool.tile([128, 1024], bf16, tag="xt")
                            nc.vector.tensor_copy(out=xt, in_=tp_bank)
                            if pend_proj is not None:
                                emit_proj(*pend_proj)
                            pend_proj = (xt, jp)
                        emit_proj(*pend_proj)

                        for cch in range(4):
                            if tname == "v":
                                skc = sq * 4 + cch
                                nc.vector.tensor_copy(
                                    out=vh_sb[:, skc, :, 0:DK],
                                    in_=prs[cch].rearrange("p (h d) -> p h d", h=8),
                                )
                            else:
                                dst = qhT_sb if tname == "q" else khT_sb
                                nc.vector.tensor_copy(
                                    out=dst[:, cch, sq * 512 : sq * 512 + 512],
                                    in_=prs[cch],
                                )

            # ---------- phase B: attention ----------
            concpool = ctx.enter_context(tc.tile_pool(name="concpool", bufs=1))
            concatT_sb = concpool.tile([128, NPAIR, S], bf16)
            with contextlib.ExitStack() as bctx:
                probpool = bctx.enter_context(tc.tile_pool(name="probpool", bufs=4))
                rppool = bctx.enter_context(tc.tile_pool(name="rppool", bufs=2))
                copool = bctx.enter_context(tc.tile_pool(name="copool", bufs=3))
                scpool = bctx.enter_context(
                    tc.tile_pool(name="scpool", bufs=2, space="PSUM")
                )
                atpool = bctx.enter_context(
                    tc.tile_pool(name="atpool", bufs=2, space="PSUM")
                )

                chunks = [
                    (sq2, pair, hh)
                    for sq2 in range(S // 1024)
                    for pair in range(NPAIR)
                    for hh in range(2)
                ]

                def emit_norm(state):
                    """Normalization for a finished head-chunk, entirely off
                    the PE: denom -> 1/denom at partition 0 (DVE), broadcast
                    to 64 partitions on the idle gpsimd engine, multiply."""
                    (sq2, pair, hh), at_ps = state
                    base = hh * 64
                    # custom-DVE recip needs partition-0 input: native copy
                    # shifts the denominator row 64 -> 0
                    den0 = rppool.tile([1, 1024], f32, tag="den0")
                    nc.vector.tensor_copy(out=den0, in_=at_ps[64:65, :])
                    recip32 = rppool.tile([1, 1024], f32, tag="recip32")
                    nc.vector.reciprocal_approx_fast(out=recip32, in_=den0)
                    # DVE can read only one PSUM operand per op: stage attn
                    attn_sb = rppool.tile([64, 1024], f32, tag="attn")
                    nc.vector.tensor_copy(out=attn_sb, in_=at_ps[0:64, :])
                    rep_sb = rppool.tile([64, 1024], f32, tag="rep")
                    nc.gpsimd.partition_broadcast(rep_sb, recip32[0:1, :])
                    nc.vector.tensor_mul(
                        concatT_sb[
                            base : base + 64, pair, sq2 * 1024 : sq2 * 1024 + 1024
                        ],
                        attn_sb,
                        rep_sb,
                    )

                def emit_cgroup(sqc, do):
                    """Out-projection group for 128 query rows x 512 out cols,
                    into a borrowed sc bank; fills PE gaps in the ACT-bound
                    attention loop (also keeps the PE p-state high)."""
                    cb = scpool.tile([128, 1024], f32, tag="sc")
                    o_ps = cb[:, 0:512]
                    for j in range(NPAIR):
                        nc.tensor.matmul(
                            o_ps,
                            lhsT=concatT_sb[:, j, sqc * 128 : sqc * 128 + 128],
                            rhs=wo_sb[:, j, do * 512 : do * 512 + 512],
                            start=(j == 0),
                            stop=(j == NPAIR - 1),
                        )
                    o_sb = copool.tile([128, 512], f32, tag="osb")
                    nc.vector.tensor_copy(out=o_sb, in_=o_ps)
                    nc.sync.dma_start(
                        out=out_d[
                            sqc * 128 : sqc * 128 + 128,
                            do * 512 : do * 512 + 512,
                        ],
                        in_=o_sb,
                    )

                pending = None      # finished chunk awaiting normalization
                for ci, (sq2, pair, hh) in enumerate(chunks):
                    h = pair * 2 + hh
                    base = hh * 64
                    at_ps = atpool.tile([128, 1024], f32, tag="at")
                    probs_q = []  # 2-deep: attn runs on probs from sk-2
                    for sk in range(NSK):
                        sc_ps = scpool.tile([128, 1024], f32, tag="sc")
                        for half in range(2):
                            qoff = sq2 * 1024 + half * 512
                            nc.tensor.matmul(
                                sc_ps[:, half * 512 : half * 512 + 512],
                                lhsT=khT_sb[
                                    base : base + 64,
                                    pair,
                                    sk * 128 : sk * 128 + 128,
                                ],
                                rhs=qhT_sb[
                                    base : base + 64, pair, qoff : qoff + 512
                                ],
                                start=True,
                                stop=True,
                            )
                        probs = probpool.tile([128, 1024], bf16, tag="probs")
                        nc.scalar.activation(
                            out=probs,
                            in_=sc_ps,
                            func=Exp,
                            bias=mb_sb[:, sk : sk + 1],
                            scale=0.125,
                        )
                        probs_q.append((sk, probs))
                        if len(probs_q) > 2:
                            psk, pp = probs_q.pop(0)
                            for half in range(2):
                                nc.tensor.matmul(
                                    at_ps[0:65, half * 512 : half * 512 + 512],
                                    lhsT=vh_sb[:, psk, h, :],
                                    rhs=pp[:, half * 512 : half * 512 + 512],
                                    start=(psk == 0),
                                    stop=False,
                                )
                        if sk == 2 and pending is not None:
                            emit_norm(pending)
                            pending = None
                    for psk, pp in probs_q:
                        for half in range(2):
                            nc.tensor.matmul(
                                at_ps[0:65, half * 512 : half * 512 + 512],
                                lhsT=vh_sb[:, psk, h, :],
                                rhs=pp[:, half * 512 : half * 512 + 512],
                                start=(psk == 0),
                                stop=(psk == NSK - 1),
                            )
                    pending = ((sq2, pair, hh), at_ps)
                emit_norm(pending)
                # out-projection tail: the PE runs these back-to-back at full
                # p-state; injecting them mid-loop would stall the in-order
                # PE queue on sc-bank reuse and starve the ACT engine
                for sqc in range(16):
                    for do in range(2):
                        emit_cgroup(sqc, do)

    nc.compile()
    return nc


def get_nc():
    if "nc" not in _cache:
        _cache["nc"] = _build_nc()
    return _cache["nc"]


def make_in_maps(q, k, v, mask, Wq, bq, Wk, bk, Wv, bv, Wo, bo):
    import ml_dtypes

    f32 = np.float32
    bf16 = ml_dtypes.bfloat16
    c = np.ascontiguousarray
    in_maps = []
    for core in range(NCORES):
        b, g = core // 2, core % 2
        cols = slice(g * CPG, (g + 1) * CPG)
        mb = (-1e9 * (1.0 - np.asarray(mask[b, 0], f32))).reshape(16, 128).T
        in_maps.append(
            {
                "q": c(np.asarray(q[b], f32).astype(bf16)),
                "k": c(np.asarray(k[b], f32).astype(bf16)),
                "v": c(np.asarray(v[b], f32).astype(bf16)),
                "wq": c(np.asarray(Wq[:, cols], f32).astype(bf16)),
                "wk": c(np.asarray(Wk[:, cols], f32).astype(bf16)),
                "wv": c(np.asarray(Wv[:, cols], f32).astype(bf16)),
                "wo": c(np.asarray(Wo[cols, :], f32).astype(bf16)),
                "bq": c(np.asarray(bq[cols], f32).astype(bf16)),
                "bk": c(np.asarray(bk[cols], f32).astype(bf16)),
                "bv": c(np.asarray(bv[cols], f32).astype(bf16)),
                "maskbias": c(mb),
                "ones": np.ones((128, 512), bf16),
                "ident": np.eye(128, dtype=bf16),
            }
        )
    return in_maps


def gather(results, bo):
    out = np.zeros((B, S, D), np.float32)
    for core in range(NCORES):
        b = core // 2
        out[b] += results[core]["out"]
    out += np.asarray(bo, np.float32)[None, None, :]
    return out


def run_on_hw(in_maps, trace=False, trace_cores=None):
    from concourse.bass_utils import run_bass_kernel_spmd

    nc = get_nc()
    return run_bass_kernel_spmd(
        nc,
        in_maps,
        list(range(NCORES)),
        trace=trace,
        trace_cores=trace_cores,
    )


def kernel(q, k, v, mask, Wq, bq, Wk, bk, Wv, bv, Wo, bo):
    in_maps = make_in_maps(q, k, v, mask, Wq, bq, Wk, bk, Wv, bv, Wo, bo)
    res = run_on_hw(in_maps)
    return gather(res.results, bo)
